# revision 21
# baseline (speedup 1.0000x reference)
"""Trainium2 Bass kernel for batch-axis-softmax attention (8 NeuronCores).

Reference computation (B=8, S=2048, D_IN=512, D_OUT=256):
    q = relu(x @ Wq + bq); k = relu(x @ Wk + bk); v = relu(x @ Wv + bv)
    scores = q @ k^T / sqrt(256)            # [B, S, S]
    attn = softmax(scores, axis=0)          # softmax over the BATCH axis
    out = attn @ v                          # [B, S, D_OUT]

Because the softmax runs over the batch axis, every (q, k) position needs
all 8 batches' scores. Two SPMD launches, no collectives:

  Launch A (batch-parallel): core b computes k^T, q^T (both [E, S]) and
  v ([S, E]) for batch b in float32r matmuls (full PE rate, near-fp32),
  emitting bf16. x arrives host-pre-transposed so the contraction dim is
  on partitions.

  Host: gathers k/v of all batches (the "all-gather"), slices q columns
  per core.

  Launch B (query-parallel): core c owns query rows [c*256, (c+1)*256) of
  EVERY batch, so the batch-axis softmax is core-local. scores^T =
  k_b @ q_slice^T in bf16 (f32 PSUM), exp on ScalarE (scores are in
  [0.18, 2.2] so no max subtraction), Z = sum_b exp, 1/Z = exp(-ln Z) on
  ScalarE (DVE RECIPROCAL measures 25.8 us), attn = exp * (1/Z) on DVE,
  out = attn^T @ v with f32 SBUF accumulation over the two k-halves.
  The work is pipelined over k-halves so the softmax join (ACT+DVE)
  overlaps the other half's PE work; ~5 us of throwaway matmuls at kernel
  start un-throttle the PE clock (HAM) before real work arrives.

exp/attn/Z/k/q/v are bf16: each elementwise bf16 error is independent
across the 2048 k positions summed by attn @ v, so it averages down by
~sqrt(2048); measured end-to-end max relative error is 2.0e-3.
"""

import numpy as np

import concourse.bacc as bacc
import concourse.mybir as mybir
import concourse.tile as tile
from concourse import bass_utils

F32 = mybir.dt.float32
F32R = mybir.dt.float32r
BF16 = mybir.dt.bfloat16

B = 8
S = 2048
D = 512
E = 256
P = 128
N_CORES = 8
QS = S // N_CORES

DC = D // P
EC = E // P
SC = S // P
SCALE = 1.0 / 16.0


def build_nc_a():
    """Projections for one batch: kt/qt [e,s] and v [s,e], all bf16."""
    nc = bacc.Bacc("TRN2", target_bir_lowering=False, debug=False,
                   num_devices=N_CORES)
    xt_d = nc.dram_tensor("xt", [D, S], F32, kind="ExternalInput")
    wq_d = nc.dram_tensor("wq", [D, E], F32, kind="ExternalInput")
    wk_d = nc.dram_tensor("wk", [D, E], F32, kind="ExternalInput")
    wv_d = nc.dram_tensor("wv", [D, E], F32, kind="ExternalInput")
    bq_d = nc.dram_tensor("bq", [E], F32, kind="ExternalInput")
    bk_d = nc.dram_tensor("bk", [E], F32, kind="ExternalInput")
    bv_d = nc.dram_tensor("bv", [E], F32, kind="ExternalInput")
    ones_d = nc.dram_tensor("onesv", [1, P], F32, kind="ExternalInput")
    kt_o = nc.dram_tensor("kt", [P, EC * S], BF16, kind="ExternalOutput")
    qt_o = nc.dram_tensor("qt", [P, EC * S], BF16, kind="ExternalOutput")
    v_o = nc.dram_tensor("v", [P, SC * E], BF16, kind="ExternalOutput")

    def mm(out, lhsT, rhs, start, stop):
        nc.tensor.matmul(out, lhsT, rhs, start=start, stop=stop)

    with tile.TileContext(nc) as tc:
        with tc.tile_pool(name="cpool", bufs=1) as cpool, \
             tc.tile_pool(name="wu", bufs=1) as wupool, \
             tc.tile_pool(name="wups", bufs=1, space="PSUM") as wups, \
             tc.tile_pool(name="p1", bufs=1) as p1pool, \
             tc.tile_pool(name="p1ps", bufs=1, space="PSUM") as p1ps:
            wu_a = wupool.tile([P, P], BF16)
            wu_b = wupool.tile([P, 512], BF16)
            nc.vector.memset(wu_a[:], 0.0)
            nc.vector.memset(wu_b[:], 0.0)
            ps_w = wups.tile([P, 512], F32)
            for i in range(24):
                nc.tensor.matmul(ps_w[:], wu_a[:], wu_b[:],
                                 start=True, stop=True)
            wq_sb = cpool.tile([P, DC, E], F32R)
            wk_sb = cpool.tile([P, DC, E], F32R)
            wv_sb = cpool.tile([P, DC, E], F32R)
            bq_sb = cpool.tile([P, EC], F32)
            bk_sb = cpool.tile([P, EC], F32)
            bv_row = cpool.tile([1, E], F32R)
            ones_row = cpool.tile([1, P], F32R)
            xt_sb = p1pool.tile([P, DC, S], F32R)
            # k weights first, then x^T chunks: the first k matmul can
            # start ~7us in instead of waiting for all 6 MB of constants
            nc.sync.dma_start(wk_sb[:], wk_d.ap().rearrange(
                "(dc p) e -> p dc e", p=P).bitcast(F32R))
            nc.sync.dma_start(bk_sb[:], bk_d.ap().rearrange(
                "(ec p) -> p ec", p=P))
            xt_r = xt_d.ap().rearrange(
                "(dc p) s -> p dc s", p=P).bitcast(F32R)
            for dc in range(DC):
                nc.sync.dma_start(xt_sb[:, dc, :], xt_r[:, dc, :])
            nc.sync.dma_start(wq_sb[:], wq_d.ap().rearrange(
                "(dc p) e -> p dc e", p=P).bitcast(F32R))
            nc.sync.dma_start(bq_sb[:], bq_d.ap().rearrange(
                "(ec p) -> p ec", p=P))
            nc.sync.dma_start(wv_sb[:], wv_d.ap().rearrange(
                "(dc p) e -> p dc e", p=P).bitcast(F32R))
            nc.sync.dma_start(bv_row[:], bv_d.ap().rearrange(
                "(a e) -> a e", a=1).bitcast(F32R))
            nc.sync.dma_start(ones_row[:], ones_d.ap().bitcast(F32R))

            # kt / qt: [e, s] = relu(W^T @ x^T + b)
            for w_sb, b_sb, o_d, nm in ((wk_sb, bk_sb, kt_o, "k"),
                                        (wq_sb, bq_sb, qt_o, "q")):
                t_sb = p1pool.tile([P, EC, S], BF16, name=f"t_{nm}")
                for ec in range(EC):
                    for sh in range(2):
                        ps_k = p1ps.tile([P, 1024], F32, tag="kps", bufs=2,
                                         name=f"ps_{nm}{ec}{sh}")
                        for dc in range(DC):
                            for st in range(2):
                                mm(ps_k[:, st * 512:(st + 1) * 512],
                                   w_sb[:, dc, ec * P:(ec + 1) * P],
                                   xt_sb[:, dc,
                                         sh * 1024 + st * 512:
                                         sh * 1024 + (st + 1) * 512],
                                   start=(dc == 0), stop=(dc == DC - 1))
                        nc.scalar.activation(
                            t_sb[:, ec, sh * 1024:(sh + 1) * 1024],
                            ps_k[:],
                            mybir.ActivationFunctionType.Relu,
                            bias=b_sb[:, ec:ec + 1])
                        nc.sync.dma_start(
                            o_d.ap().rearrange(
                                "p (ec s) -> p ec s", ec=EC)
                            [:, ec, sh * 1024:(sh + 1) * 1024],
                            t_sb[:, ec, sh * 1024:(sh + 1) * 1024])

            # v: [s, e] = relu(x @ Wv + bv), bias via rank-1 matmul
            v_sb = p1pool.tile([P, SC * E], BF16)
            for sp in range(SC // 2):
                ps_v = p1ps.tile([P, 2 * E], F32, tag="vps", bufs=2)
                for half in range(2):
                    st = sp * 2 + half
                    sl = ps_v[:, half * E:(half + 1) * E]
                    mm(sl, ones_row[0:1, :], bv_row[0:1, :],
                       start=True, stop=False)
                    for dc in range(DC):
                        mm(sl, xt_sb[:, dc, st * P:(st + 1) * P],
                           wv_sb[:, dc, :],
                           start=False, stop=(dc == DC - 1))
                nc.scalar.activation(
                    v_sb[:, sp * 2 * E:(sp + 1) * 2 * E], ps_v[:],
                    mybir.ActivationFunctionType.Relu)
                nc.sync.dma_start(
                    v_o.ap()[:, sp * 2 * E:(sp + 1) * 2 * E],
                    v_sb[:, sp * 2 * E:(sp + 1) * 2 * E])

    nc.compile()
    return nc


def build_nc_b():
    """Attention for one q-slice of 256 rows, all batches.

    Pipelined over the two k-halves: while half 1's scores stream on the
    PE, half 0's Z/R/attn (ACT+DVE) and combine run, and vice versa —
    the softmax join never idles the PE. Output accumulates in SBUF f32
    across the two halves.
    """
    nc = bacc.Bacc("TRN2", target_bir_lowering=False, debug=False,
                   num_devices=N_CORES)
    kt_d = nc.dram_tensor("ktall", [B, P, EC * S], BF16,
                          kind="ExternalInput")
    v_d = nc.dram_tensor("vall", [B, P, SC * E], BF16,
                         kind="ExternalInput")
    qsl_d = nc.dram_tensor("qsl", [P, B, EC, QS], BF16,
                           kind="ExternalInput")
    out_d = nc.dram_tensor("out", [B, QS, E], F32, kind="ExternalOutput")

    HS = S // 2          # 1024 columns of k per half
    HC = SC // 2         # 8 kpos chunks per half

    def mm(out, lhsT, rhs, start, stop):
        nc.tensor.matmul(out, lhsT, rhs, start=start, stop=stop)

    with tile.TileContext(nc) as tc:
        with tc.tile_pool(name="p2", bufs=1) as p2pool, \
             tc.tile_pool(name="kstream", bufs=6) as kstream, \
             tc.tile_pool(name="vstream", bufs=4) as vstream, \
             tc.tile_pool(name="wu", bufs=1) as wupool:

            qsl_sb = p2pool.tile([P, B, EC, QS], BF16)
            for b in range(B):
                nc.gpsimd.dma_start(qsl_sb[:, b], qsl_d.ap()[:, b])

            exp_all = p2pool.tile([P, B, SC, QS], BF16)
            z_sb = p2pool.tile([P, SC, QS], BF16)
            r_sb = p2pool.tile([P, SC, QS], BF16)

            kt_v = kt_d.ap().rearrange("b p (ec s) -> b p ec s", ec=EC)
            v_v = v_d.ap().rearrange("b p (st e) -> b p st e", st=SC)

            # ---- scores + exp + Z + attn, pipelined over k-halves ----
            # All scores (both halves) stream on the PE back-to-back; each
            # half's softmax join (Z tail, 1/Z on ACT, attn muls on DVE)
            # overlaps the other half's PE work.
            with tc.tile_pool(name="sps", bufs=1, space="PSUM") as spspool:
                # PE warm-up: ~5us of throwaway matmuls during the head
                # DMAs so the HAM un-throttles (1.2 -> 2.4 GHz).
                wu_a = wupool.tile([P, P], BF16)
                wu_b = wupool.tile([P, E], BF16)
                nc.vector.memset(wu_a[:], 0.0)
                nc.vector.memset(wu_b[:], 0.0)
                for i in range(24):
                    ps_w = spspool.tile([P, HC, QS], F32, tag="sps", bufs=2,
                                        name=f"ps_w{i}")
                    nc.tensor.matmul(ps_w[:, 0, :E], wu_a[:], wu_b[:],
                                     start=True, stop=True)

                for half in range(2):
                    for b in range(B):
                        kt_h = kstream.tile([P, EC, HS], BF16, tag="kt",
                                            name=f"kt_{half}_{b}")
                        nc.sync.dma_start(
                            kt_h[:],
                            kt_v[b, :, :, half * HS:(half + 1) * HS])
                        ps_s = spspool.tile([P, HC, QS], F32, tag="sps",
                                            bufs=2, name=f"ps_s{half}_{b}")
                        for kc8 in range(HC):
                            for ec in range(EC):
                                mm(ps_s[:, kc8, :],
                                   kt_h[:, ec, kc8 * P:(kc8 + 1) * P],
                                   qsl_sb[:, b, ec, :],
                                   start=(ec == 0), stop=(ec == EC - 1))
                        nc.scalar.activation(
                            exp_all[:, b, half * HC:(half + 1) * HC, :],
                            ps_s[:],
                            mybir.ActivationFunctionType.Exp,
                            scale=SCALE)
                        zh = z_sb[:, half * HC:(half + 1) * HC, :]
                        eh = exp_all[:, b, half * HC:(half + 1) * HC, :]
                        if b == 0:
                            nc.vector.tensor_copy(zh, eh)
                        else:
                            nc.vector.tensor_add(zh, zh, eh)

                    # 1/Z = exp(-ln Z) on ScalarE (DVE RECIPROCAL is
                    # 25.8us on this tile), then attn = exp * R on DVE
                    rh = r_sb[:, half * HC:(half + 1) * HC, :]
                    nc.scalar.activation(
                        rh, z_sb[:, half * HC:(half + 1) * HC, :],
                        mybir.ActivationFunctionType.Ln)
                    nc.scalar.activation(
                        rh, rh, mybir.ActivationFunctionType.Exp,
                        scale=-1.0)
                    for b in range(B):
                        eh = exp_all[:, b, half * HC:(half + 1) * HC, :]
                        nc.vector.tensor_mul(eh, eh, rh)

            # ---- combine: out = attn^T @ v ----
            # runs after all scores/joins, so each (b, qc) accumulates all
            # 16 k-chunks in one PSUM group; output copies once via DVE
            with tc.tile_pool(name="ops", bufs=1, space="PSUM") as opspool, \
                 tc.tile_pool(name="outp", bufs=4) as outpool:
                for b in range(B):
                    v_b = vstream.tile([P, SC, E], BF16, tag="v",
                                       name=f"v_{b}")
                    nc.gpsimd.dma_start(v_b[:], v_v[b])
                    for qc in range(2):
                        ps_o = opspool.tile([P, E], F32, tag="ops",
                                            bufs=8, name=f"ps_o{b}_{qc}")
                        for st in range(SC):
                            nc.tensor.matmul(
                                ps_o[:],
                                exp_all[:, b, st, qc * P:(qc + 1) * P],
                                v_b[:, st, :],
                                start=(st == 0), stop=(st == SC - 1))
                        o_sb = outpool.tile([P, E], F32, tag="osb",
                                            name=f"o_sb{b}_{qc}")
                        nc.vector.tensor_copy(o_sb[:], ps_o[:])
                        nc.sync.dma_start(
                            out_d.ap()[b, qc * P:(qc + 1) * P, :], o_sb[:])

    nc.compile()
    return nc


_CACHE = {}


def get_nc(which):
    if which not in _CACHE:
        _CACHE[which] = build_nc_a() if which == "a" else build_nc_b()
    return _CACHE[which]


def make_in_maps_a(x, Wq, bq, Wk, bk, Wv, bv):
    ones = np.ones((1, P), np.float32)
    maps = []
    for c in range(N_CORES):
        xt = np.ascontiguousarray(x[c].T)
        maps.append({"xt": xt, "wq": Wq, "wk": Wk, "wv": Wv,
                     "bq": bq, "bk": bk, "bv": bv, "onesv": ones})
    return maps


def make_in_maps_b(res_a):
    ktall = np.stack([res_a[b]["kt"] for b in range(B)])  # [B,P,EC*S]
    vall = np.stack([res_a[b]["v"] for b in range(B)])
    # qt_b [P, EC*S] -> [P, EC, S]; core c needs columns c*QS..
    qts = [res_a[b]["qt"].reshape(P, EC, S) for b in range(B)]
    maps = []
    for c in range(N_CORES):
        qsl = np.stack([q[:, :, c * QS:(c + 1) * QS] for q in qts],
                       axis=1)  # [P, B, EC, QS]
        maps.append({"ktall": ktall, "vall": vall,
                     "qsl": np.ascontiguousarray(qsl)})
    return maps


def run(x, Wq, bq, Wk, bk, Wv, bv, trace=False):
    nc_a = get_nc("a")
    nc_b = get_nc("b")
    ra = bass_utils.run_bass_kernel_spmd(
        nc_a, make_in_maps_a(x, Wq, bq, Wk, bk, Wv, bv),
        core_ids=list(range(N_CORES)), trace=trace)
    rb = bass_utils.run_bass_kernel_spmd(
        nc_b, make_in_maps_b(ra.results),
        core_ids=list(range(N_CORES)), trace=trace)
    out = np.empty((B, S, E), np.float32)
    for c in range(N_CORES):
        out[:, c * QS:(c + 1) * QS, :] = rb.results[c]["out"]
    return out, ra, rb


def kernel(x, Wq, bq, Wk, bk, Wv, bv):
    out, _, _ = run(np.asarray(x, np.float32),
                    np.asarray(Wq, np.float32), np.asarray(bq, np.float32),
                    np.asarray(Wk, np.float32), np.asarray(bk, np.float32),
                    np.asarray(Wv, np.float32), np.asarray(bv, np.float32))
    return out


# revision 22
# speedup vs baseline: 1.0377x; 1.0377x over previous
"""Trainium2 Bass kernel for batch-axis-softmax attention (8 NeuronCores).

Reference computation (B=8, S=2048, D_IN=512, D_OUT=256):
    q = relu(x @ Wq + bq); k = relu(x @ Wk + bk); v = relu(x @ Wv + bv)
    scores = q @ k^T / sqrt(256)            # [B, S, S]
    attn = softmax(scores, axis=0)          # softmax over the BATCH axis
    out = attn @ v                          # [B, S, D_OUT]

Because the softmax runs over the batch axis, every (q, k) position needs
all 8 batches' scores. Two SPMD launches, no collectives:

  Launch A (batch-parallel): core b computes k^T, q^T (both [E, S]) and
  v ([S, E]) for batch b in float32r matmuls (full PE rate, near-fp32),
  emitting bf16. x arrives host-pre-transposed so the contraction dim is
  on partitions.

  Host: gathers k/v of all batches (the "all-gather"), slices q columns
  per core.

  Launch B (query-parallel): core c owns query rows [c*256, (c+1)*256) of
  EVERY batch, so the batch-axis softmax is core-local. scores^T =
  k_b @ q_slice^T in bf16 (f32 PSUM), exp on ScalarE (scores are in
  [0.18, 2.2] so no max subtraction), Z = sum_b exp, 1/Z = exp(-ln Z) on
  ScalarE (DVE RECIPROCAL measures 25.8 us), attn = exp * (1/Z) on DVE,
  out = attn^T @ v with f32 SBUF accumulation over the two k-halves.
  The work is pipelined over k-halves so the softmax join (ACT+DVE)
  overlaps the other half's PE work; ~5 us of throwaway matmuls at kernel
  start un-throttle the PE clock (HAM) before real work arrives.

exp/attn/Z/k/q/v are bf16: each elementwise bf16 error is independent
across the 2048 k positions summed by attn @ v, so it averages down by
~sqrt(2048); measured end-to-end max relative error is 2.0e-3.
"""

import numpy as np

import concourse.bacc as bacc
import concourse.mybir as mybir
import concourse.tile as tile
from concourse import bass_utils

F32 = mybir.dt.float32
F32R = mybir.dt.float32r
BF16 = mybir.dt.bfloat16

B = 8
S = 2048
D = 512
E = 256
P = 128
N_CORES = 8
QS = S // N_CORES

DC = D // P
EC = E // P
SC = S // P
SCALE = 1.0 / 16.0


def build_nc_a():
    """Projections for one batch: kt/qt [e,s] and v [s,e], all bf16."""
    nc = bacc.Bacc("TRN2", target_bir_lowering=False, debug=False,
                   num_devices=N_CORES)
    xt_d = nc.dram_tensor("xt", [D, S], F32, kind="ExternalInput")
    wq_d = nc.dram_tensor("wq", [D, E], F32, kind="ExternalInput")
    wk_d = nc.dram_tensor("wk", [D, E], F32, kind="ExternalInput")
    wv_d = nc.dram_tensor("wv", [D, E], F32, kind="ExternalInput")
    bq_d = nc.dram_tensor("bq", [E], F32, kind="ExternalInput")
    bk_d = nc.dram_tensor("bk", [E], F32, kind="ExternalInput")
    bv_d = nc.dram_tensor("bv", [E], F32, kind="ExternalInput")
    ones_d = nc.dram_tensor("onesv", [1, P], F32, kind="ExternalInput")
    kt_o = nc.dram_tensor("kt", [P, EC * S], BF16, kind="ExternalOutput")
    qt_o = nc.dram_tensor("qt", [P, EC * S], BF16, kind="ExternalOutput")
    v_o = nc.dram_tensor("v", [P, SC * E], BF16, kind="ExternalOutput")

    def mm(out, lhsT, rhs, start, stop):
        nc.tensor.matmul(out, lhsT, rhs, start=start, stop=stop)

    with tile.TileContext(nc) as tc:
        with tc.tile_pool(name="cpool", bufs=1) as cpool, \
             tc.tile_pool(name="wu", bufs=1) as wupool, \
             tc.tile_pool(name="wups", bufs=1, space="PSUM") as wups, \
             tc.tile_pool(name="p1", bufs=1) as p1pool, \
             tc.tile_pool(name="p1ps", bufs=1, space="PSUM") as p1ps:
            wu_a = wupool.tile([P, P], BF16)
            wu_b = wupool.tile([P, 512], BF16)
            nc.vector.memset(wu_a[:], 0.0)
            nc.vector.memset(wu_b[:], 0.0)
            ps_w = wups.tile([P, 512], F32)
            for i in range(24):
                nc.tensor.matmul(ps_w[:], wu_a[:], wu_b[:],
                                 start=True, stop=True)
            wq_sb = cpool.tile([P, DC, E], F32R)
            wk_sb = cpool.tile([P, DC, E], F32R)
            wv_sb = cpool.tile([P, DC, E], F32R)
            bq_sb = cpool.tile([P, EC], F32)
            bk_sb = cpool.tile([P, EC], F32)
            bv_row = cpool.tile([1, E], F32R)
            ones_row = cpool.tile([1, P], F32R)
            xt_sb = p1pool.tile([P, DC, S], F32R)
            # k weights first, then x^T chunks: the first k matmul can
            # start ~7us in instead of waiting for all 6 MB of constants
            nc.sync.dma_start(wk_sb[:], wk_d.ap().rearrange(
                "(dc p) e -> p dc e", p=P).bitcast(F32R))
            nc.sync.dma_start(bk_sb[:], bk_d.ap().rearrange(
                "(ec p) -> p ec", p=P))
            xt_r = xt_d.ap().rearrange(
                "(dc p) s -> p dc s", p=P).bitcast(F32R)
            for dc in range(DC):
                nc.sync.dma_start(xt_sb[:, dc, :], xt_r[:, dc, :])
            nc.sync.dma_start(wq_sb[:], wq_d.ap().rearrange(
                "(dc p) e -> p dc e", p=P).bitcast(F32R))
            nc.sync.dma_start(bq_sb[:], bq_d.ap().rearrange(
                "(ec p) -> p ec", p=P))
            nc.sync.dma_start(wv_sb[:], wv_d.ap().rearrange(
                "(dc p) e -> p dc e", p=P).bitcast(F32R))
            nc.sync.dma_start(bv_row[:], bv_d.ap().rearrange(
                "(a e) -> a e", a=1).bitcast(F32R))
            nc.sync.dma_start(ones_row[:], ones_d.ap().bitcast(F32R))

            # kt / qt: [e, s] = relu(W^T @ x^T + b)
            for w_sb, b_sb, o_d, nm in ((wk_sb, bk_sb, kt_o, "k"),
                                        (wq_sb, bq_sb, qt_o, "q")):
                t_sb = p1pool.tile([P, EC, S], BF16, name=f"t_{nm}")
                for ec in range(EC):
                    for sh in range(2):
                        ps_k = p1ps.tile([P, 1024], F32, tag="kps", bufs=2,
                                         name=f"ps_{nm}{ec}{sh}")
                        for dc in range(DC):
                            for st in range(2):
                                mm(ps_k[:, st * 512:(st + 1) * 512],
                                   w_sb[:, dc, ec * P:(ec + 1) * P],
                                   xt_sb[:, dc,
                                         sh * 1024 + st * 512:
                                         sh * 1024 + (st + 1) * 512],
                                   start=(dc == 0), stop=(dc == DC - 1))
                        nc.scalar.activation(
                            t_sb[:, ec, sh * 1024:(sh + 1) * 1024],
                            ps_k[:],
                            mybir.ActivationFunctionType.Relu,
                            bias=b_sb[:, ec:ec + 1])
                        nc.sync.dma_start(
                            o_d.ap().rearrange(
                                "p (ec s) -> p ec s", ec=EC)
                            [:, ec, sh * 1024:(sh + 1) * 1024],
                            t_sb[:, ec, sh * 1024:(sh + 1) * 1024])

            # v: [s, e] = relu(x @ Wv + bv), bias via rank-1 matmul
            v_sb = p1pool.tile([P, SC * E], BF16)
            for sp in range(SC // 2):
                ps_v = p1ps.tile([P, 2 * E], F32, tag="vps", bufs=2)
                for half in range(2):
                    st = sp * 2 + half
                    sl = ps_v[:, half * E:(half + 1) * E]
                    mm(sl, ones_row[0:1, :], bv_row[0:1, :],
                       start=True, stop=False)
                    for dc in range(DC):
                        mm(sl, xt_sb[:, dc, st * P:(st + 1) * P],
                           wv_sb[:, dc, :],
                           start=False, stop=(dc == DC - 1))
                nc.scalar.activation(
                    v_sb[:, sp * 2 * E:(sp + 1) * 2 * E], ps_v[:],
                    mybir.ActivationFunctionType.Relu)
                nc.sync.dma_start(
                    v_o.ap()[:, sp * 2 * E:(sp + 1) * 2 * E],
                    v_sb[:, sp * 2 * E:(sp + 1) * 2 * E])

    nc.compile()
    return nc


def build_nc_b():
    """Attention for one q-slice of 256 rows, all batches.

    Pipelined over the two k-halves: while half 1's scores stream on the
    PE, half 0's Z/R/attn (ACT+DVE) and combine run, and vice versa —
    the softmax join never idles the PE. Output accumulates in SBUF f32
    across the two halves.
    """
    nc = bacc.Bacc("TRN2", target_bir_lowering=False, debug=False,
                   num_devices=N_CORES)
    kt_d = nc.dram_tensor("ktall", [B, P, EC * S], BF16,
                          kind="ExternalInput")
    v_d = nc.dram_tensor("vall", [B, P, SC * E], BF16,
                         kind="ExternalInput")
    qsl_d = nc.dram_tensor("qsl", [P, B, EC, QS], BF16,
                           kind="ExternalInput")
    out_d = nc.dram_tensor("out", [B, QS, E], F32, kind="ExternalOutput")

    HS = S // 2          # 1024 columns of k per half
    HC = SC // 2         # 8 kpos chunks per half

    def mm(out, lhsT, rhs, start, stop):
        nc.tensor.matmul(out, lhsT, rhs, start=start, stop=stop)

    with tile.TileContext(nc) as tc:
        with tc.tile_pool(name="p2", bufs=1) as p2pool, \
             tc.tile_pool(name="kstream", bufs=4) as kstream, \
             tc.tile_pool(name="vstream", bufs=4) as vstream, \
             tc.tile_pool(name="wu", bufs=1) as wupool:

            qsl_sb = p2pool.tile([P, B, EC, QS], BF16)
            nc.gpsimd.dma_start(qsl_sb[:], qsl_d.ap())

            exp_all = p2pool.tile([P, B, SC, QS], BF16)
            z_sb = p2pool.tile([P, SC, QS], BF16)
            r_sb = p2pool.tile([P, SC, QS], BF16)

            kt_v = kt_d.ap().rearrange("b p (ec s) -> b p ec s", ec=EC)
            v_v = v_d.ap().rearrange("b p (st e) -> b p st e", st=SC)

            # ---- scores + exp + Z + attn, pipelined over k-halves ----
            # All scores (both halves) stream on the PE back-to-back; each
            # half's softmax join (Z tail, 1/Z on ACT, attn muls on DVE)
            # overlaps the other half's PE work.
            with tc.tile_pool(name="sps", bufs=1, space="PSUM") as spspool:
                # PE warm-up: ~5us of throwaway matmuls during the head
                # DMAs so the HAM un-throttles (1.2 -> 2.4 GHz).
                wu_a = wupool.tile([P, P], BF16)
                wu_b = wupool.tile([P, E], BF16)
                nc.vector.memset(wu_a[:], 0.0)
                nc.vector.memset(wu_b[:], 0.0)
                for i in range(24):
                    ps_w = spspool.tile([P, HC, QS], F32, tag="sps", bufs=2,
                                        name=f"ps_w{i}")
                    nc.tensor.matmul(ps_w[:, 0, :E], wu_a[:], wu_b[:],
                                     start=True, stop=True)

                for half in range(2):
                    for b in range(B):
                        kt_h = kstream.tile([P, EC, HS], BF16, tag="kt",
                                            name=f"kt_{half}_{b}")
                        nc.sync.dma_start(
                            kt_h[:],
                            kt_v[b, :, :, half * HS:(half + 1) * HS])
                        ps_s = spspool.tile([P, HC, QS], F32, tag="sps",
                                            bufs=2, name=f"ps_s{half}_{b}")
                        for kc8 in range(HC):
                            for ec in range(EC):
                                mm(ps_s[:, kc8, :],
                                   kt_h[:, ec, kc8 * P:(kc8 + 1) * P],
                                   qsl_sb[:, b, ec, :],
                                   start=(ec == 0), stop=(ec == EC - 1))
                        nc.scalar.activation(
                            exp_all[:, b, half * HC:(half + 1) * HC, :],
                            ps_s[:],
                            mybir.ActivationFunctionType.Exp,
                            scale=SCALE)
                        zh = z_sb[:, half * HC:(half + 1) * HC, :]
                        eh = exp_all[:, b, half * HC:(half + 1) * HC, :]
                        if b == 0:
                            nc.vector.tensor_copy(zh, eh)
                        else:
                            nc.vector.tensor_add(zh, zh, eh)

                    # 1/Z = exp(-ln Z) on ScalarE (DVE RECIPROCAL is
                    # 25.8us on this tile), then attn = exp * R on DVE
                    rh = r_sb[:, half * HC:(half + 1) * HC, :]
                    nc.scalar.activation(
                        rh, z_sb[:, half * HC:(half + 1) * HC, :],
                        mybir.ActivationFunctionType.Ln)
                    nc.scalar.activation(
                        rh, rh, mybir.ActivationFunctionType.Exp,
                        scale=-1.0)
                    for b in range(B):
                        eh = exp_all[:, b, half * HC:(half + 1) * HC, :]
                        nc.vector.tensor_mul(eh, eh, rh)

            # ---- combine: out = attn^T @ v ----
            # runs after all scores/joins, so each (b, qc) accumulates all
            # 16 k-chunks in one PSUM group; output copies once via DVE
            with tc.tile_pool(name="ops", bufs=1, space="PSUM") as opspool, \
                 tc.tile_pool(name="outp", bufs=4) as outpool:
                for b in range(B):
                    v_b = vstream.tile([P, SC, E], BF16, tag="v",
                                       name=f"v_{b}")
                    nc.gpsimd.dma_start(v_b[:], v_v[b])
                    for qc in range(2):
                        ps_o = opspool.tile([P, E], F32, tag="ops",
                                            bufs=8, name=f"ps_o{b}_{qc}")
                        for st in range(SC):
                            nc.tensor.matmul(
                                ps_o[:],
                                exp_all[:, b, st, qc * P:(qc + 1) * P],
                                v_b[:, st, :],
                                start=(st == 0), stop=(st == SC - 1))
                        o_sb = outpool.tile([P, E], F32, tag="osb",
                                            name=f"o_sb{b}_{qc}")
                        nc.vector.tensor_copy(o_sb[:], ps_o[:])
                        nc.sync.dma_start(
                            out_d.ap()[b, qc * P:(qc + 1) * P, :], o_sb[:])

    nc.compile()
    return nc


_CACHE = {}


def get_nc(which):
    if which not in _CACHE:
        _CACHE[which] = build_nc_a() if which == "a" else build_nc_b()
    return _CACHE[which]


def make_in_maps_a(x, Wq, bq, Wk, bk, Wv, bv):
    ones = np.ones((1, P), np.float32)
    maps = []
    for c in range(N_CORES):
        xt = np.ascontiguousarray(x[c].T)
        maps.append({"xt": xt, "wq": Wq, "wk": Wk, "wv": Wv,
                     "bq": bq, "bk": bk, "bv": bv, "onesv": ones})
    return maps


def make_in_maps_b(res_a):
    ktall = np.stack([res_a[b]["kt"] for b in range(B)])  # [B,P,EC*S]
    vall = np.stack([res_a[b]["v"] for b in range(B)])
    # qt_b [P, EC*S] -> [P, EC, S]; core c needs columns c*QS..
    qts = [res_a[b]["qt"].reshape(P, EC, S) for b in range(B)]
    maps = []
    for c in range(N_CORES):
        qsl = np.stack([q[:, :, c * QS:(c + 1) * QS] for q in qts],
                       axis=1)  # [P, B, EC, QS]
        maps.append({"ktall": ktall, "vall": vall,
                     "qsl": np.ascontiguousarray(qsl)})
    return maps


def run(x, Wq, bq, Wk, bk, Wv, bv, trace=False):
    nc_a = get_nc("a")
    nc_b = get_nc("b")
    ra = bass_utils.run_bass_kernel_spmd(
        nc_a, make_in_maps_a(x, Wq, bq, Wk, bk, Wv, bv),
        core_ids=list(range(N_CORES)), trace=trace)
    rb = bass_utils.run_bass_kernel_spmd(
        nc_b, make_in_maps_b(ra.results),
        core_ids=list(range(N_CORES)), trace=trace)
    out = np.empty((B, S, E), np.float32)
    for c in range(N_CORES):
        out[:, c * QS:(c + 1) * QS, :] = rb.results[c]["out"]
    return out, ra, rb


def kernel(x, Wq, bq, Wk, bk, Wv, bv):
    out, _, _ = run(np.asarray(x, np.float32),
                    np.asarray(Wq, np.float32), np.asarray(bq, np.float32),
                    np.asarray(Wk, np.float32), np.asarray(bk, np.float32),
                    np.asarray(Wv, np.float32), np.asarray(bv, np.float32))
    return out


# revision 23
# speedup vs baseline: 1.0705x; 1.0316x over previous
"""Trainium2 Bass kernel for batch-axis-softmax attention (8 NeuronCores).

Reference computation (B=8, S=2048, D_IN=512, D_OUT=256):
    q = relu(x @ Wq + bq); k = relu(x @ Wk + bk); v = relu(x @ Wv + bv)
    scores = q @ k^T / sqrt(256)            # [B, S, S]
    attn = softmax(scores, axis=0)          # softmax over the BATCH axis
    out = attn @ v                          # [B, S, D_OUT]

Because the softmax runs over the batch axis, every (q, k) position needs
all 8 batches' scores. Two SPMD launches, no collectives:

  Launch A (batch-parallel): core b computes k^T, q^T (both [E, S]) and
  v ([S, E]) for batch b in float32r matmuls (full PE rate, near-fp32),
  emitting bf16. x arrives host-pre-transposed so the contraction dim is
  on partitions.

  Host: gathers k/v of all batches (the "all-gather"), slices q columns
  per core.

  Launch B (query-parallel): core c owns query rows [c*256, (c+1)*256) of
  EVERY batch, so the batch-axis softmax is core-local. scores^T =
  k_b @ q_slice^T in bf16 (f32 PSUM), exp on ScalarE (scores are in
  [0.18, 2.2] so no max subtraction), Z = sum_b exp, 1/Z = exp(-ln Z) on
  ScalarE (DVE RECIPROCAL measures 25.8 us), attn = exp * (1/Z) on DVE,
  out = attn^T @ v with f32 SBUF accumulation over the two k-halves.
  The work is pipelined over k-halves so the softmax join (ACT+DVE)
  overlaps the other half's PE work; ~5 us of throwaway matmuls at kernel
  start un-throttle the PE clock (HAM) before real work arrives.

exp/attn/Z/k/q/v are bf16: each elementwise bf16 error is independent
across the 2048 k positions summed by attn @ v, so it averages down by
~sqrt(2048); measured end-to-end max relative error is 2.0e-3.
"""

import numpy as np

import concourse.bacc as bacc
import concourse.mybir as mybir
import concourse.tile as tile
from concourse import bass_utils

F32 = mybir.dt.float32
F32R = mybir.dt.float32r
BF16 = mybir.dt.bfloat16

B = 8
S = 2048
D = 512
E = 256
P = 128
N_CORES = 8
QS = S // N_CORES

DC = D // P
EC = E // P
SC = S // P
SCALE = 1.0 / 16.0


def build_nc_a():
    """Projections for one batch: kt/qt [e,s] and v [s,e], all bf16."""
    nc = bacc.Bacc("TRN2", target_bir_lowering=False, debug=False,
                   num_devices=N_CORES)
    xt_d = nc.dram_tensor("xt", [D, S], F32, kind="ExternalInput")
    wq_d = nc.dram_tensor("wq", [D, E], F32, kind="ExternalInput")
    wk_d = nc.dram_tensor("wk", [D, E], F32, kind="ExternalInput")
    wv_d = nc.dram_tensor("wv", [D, E], F32, kind="ExternalInput")
    bq_d = nc.dram_tensor("bq", [E], F32, kind="ExternalInput")
    bk_d = nc.dram_tensor("bk", [E], F32, kind="ExternalInput")
    bv_d = nc.dram_tensor("bv", [E], F32, kind="ExternalInput")
    ones_d = nc.dram_tensor("onesv", [1, P], F32, kind="ExternalInput")
    kt_o = nc.dram_tensor("kt", [P, EC * S], BF16, kind="ExternalOutput")
    qt_o = nc.dram_tensor("qt", [P, EC * S], BF16, kind="ExternalOutput")
    v_o = nc.dram_tensor("v", [P, SC * E], BF16, kind="ExternalOutput")

    def mm(out, lhsT, rhs, start, stop):
        nc.tensor.matmul(out, lhsT, rhs, start=start, stop=stop)

    with tile.TileContext(nc) as tc:
        with tc.tile_pool(name="cpool", bufs=1) as cpool, \
             tc.tile_pool(name="wu", bufs=1) as wupool, \
             tc.tile_pool(name="wups", bufs=1, space="PSUM") as wups, \
             tc.tile_pool(name="p1", bufs=1) as p1pool, \
             tc.tile_pool(name="p1ps", bufs=1, space="PSUM") as p1ps:
            wu_a = wupool.tile([P, P], BF16)
            wu_b = wupool.tile([P, 512], BF16)
            nc.vector.memset(wu_a[:], 0.0)
            nc.vector.memset(wu_b[:], 0.0)
            ps_w = wups.tile([P, 512], F32)
            for i in range(24):
                nc.tensor.matmul(ps_w[:], wu_a[:], wu_b[:],
                                 start=True, stop=True)
            wq_sb = cpool.tile([P, DC, E], F32R)
            wk_sb = cpool.tile([P, DC, E], F32R)
            wv_sb = cpool.tile([P, DC, E], F32R)
            bq_sb = cpool.tile([P, EC], F32)
            bk_sb = cpool.tile([P, EC], F32)
            bv_row = cpool.tile([1, E], F32R)
            ones_row = cpool.tile([1, P], F32R)
            xt_sb = p1pool.tile([P, DC, S], F32R)
            # k weights first, then x^T chunks: the first k matmul can
            # start ~7us in instead of waiting for all 6 MB of constants
            nc.sync.dma_start(wk_sb[:], wk_d.ap().rearrange(
                "(dc p) e -> p dc e", p=P).bitcast(F32R))
            nc.sync.dma_start(bk_sb[:], bk_d.ap().rearrange(
                "(ec p) -> p ec", p=P))
            xt_r = xt_d.ap().rearrange(
                "(dc p) s -> p dc s", p=P).bitcast(F32R)
            for dc in range(DC):
                nc.sync.dma_start(xt_sb[:, dc, :], xt_r[:, dc, :])
            nc.sync.dma_start(wq_sb[:], wq_d.ap().rearrange(
                "(dc p) e -> p dc e", p=P).bitcast(F32R))
            nc.sync.dma_start(bq_sb[:], bq_d.ap().rearrange(
                "(ec p) -> p ec", p=P))
            nc.sync.dma_start(wv_sb[:], wv_d.ap().rearrange(
                "(dc p) e -> p dc e", p=P).bitcast(F32R))
            nc.sync.dma_start(bv_row[:], bv_d.ap().rearrange(
                "(a e) -> a e", a=1).bitcast(F32R))
            nc.sync.dma_start(ones_row[:], ones_d.ap().bitcast(F32R))

            # kt / qt: [e, s] = relu(W^T @ x^T + b)
            for w_sb, b_sb, o_d, nm in ((wk_sb, bk_sb, kt_o, "k"),
                                        (wq_sb, bq_sb, qt_o, "q")):
                t_sb = p1pool.tile([P, EC, S], BF16, name=f"t_{nm}")
                for ec in range(EC):
                    for sh in range(2):
                        ps_k = p1ps.tile([P, 1024], F32, tag="kps", bufs=2,
                                         name=f"ps_{nm}{ec}{sh}")
                        for dc in range(DC):
                            for st in range(2):
                                mm(ps_k[:, st * 512:(st + 1) * 512],
                                   w_sb[:, dc, ec * P:(ec + 1) * P],
                                   xt_sb[:, dc,
                                         sh * 1024 + st * 512:
                                         sh * 1024 + (st + 1) * 512],
                                   start=(dc == 0), stop=(dc == DC - 1))
                        nc.scalar.activation(
                            t_sb[:, ec, sh * 1024:(sh + 1) * 1024],
                            ps_k[:],
                            mybir.ActivationFunctionType.Relu,
                            bias=b_sb[:, ec:ec + 1])
                        nc.sync.dma_start(
                            o_d.ap().rearrange(
                                "p (ec s) -> p ec s", ec=EC)
                            [:, ec, sh * 1024:(sh + 1) * 1024],
                            t_sb[:, ec, sh * 1024:(sh + 1) * 1024])

            # v: [s, e] = relu(x @ Wv + bv), bias via rank-1 matmul
            v_sb = p1pool.tile([P, SC * E], BF16)
            for sp in range(SC // 2):
                ps_v = p1ps.tile([P, 2 * E], F32, tag="vps", bufs=2)
                for half in range(2):
                    st = sp * 2 + half
                    sl = ps_v[:, half * E:(half + 1) * E]
                    mm(sl, ones_row[0:1, :], bv_row[0:1, :],
                       start=True, stop=False)
                    for dc in range(DC):
                        mm(sl, xt_sb[:, dc, st * P:(st + 1) * P],
                           wv_sb[:, dc, :],
                           start=False, stop=(dc == DC - 1))
                nc.scalar.activation(
                    v_sb[:, sp * 2 * E:(sp + 1) * 2 * E], ps_v[:],
                    mybir.ActivationFunctionType.Relu)
                nc.sync.dma_start(
                    v_o.ap()[:, sp * 2 * E:(sp + 1) * 2 * E],
                    v_sb[:, sp * 2 * E:(sp + 1) * 2 * E])

    nc.compile()
    return nc


def build_nc_b():
    """Attention for one q-slice of 256 rows, all batches.

    Pipelined over the two k-halves: while half 1's scores stream on the
    PE, half 0's Z/R/attn (ACT+DVE) and combine run, and vice versa —
    the softmax join never idles the PE. Output accumulates in SBUF f32
    across the two halves.
    """
    nc = bacc.Bacc("TRN2", target_bir_lowering=False, debug=False,
                   num_devices=N_CORES)
    kt_d = nc.dram_tensor("ktall", [B, P, EC * S], BF16,
                          kind="ExternalInput")
    v_d = nc.dram_tensor("vall", [B, P, SC * E], BF16,
                         kind="ExternalInput")
    qsl_d = nc.dram_tensor("qsl", [P, B, EC, QS], BF16,
                           kind="ExternalInput")
    out_d = nc.dram_tensor("out", [B, QS, E], F32, kind="ExternalOutput")

    HS = S // 2          # 1024 columns of k per half
    HC = SC // 2         # 8 kpos chunks per half

    def mm(out, lhsT, rhs, start, stop):
        nc.tensor.matmul(out, lhsT, rhs, start=start, stop=stop)

    with tile.TileContext(nc) as tc:
        with tc.tile_pool(name="p2", bufs=1) as p2pool, \
             tc.tile_pool(name="kstream", bufs=4) as kstream, \
             tc.tile_pool(name="vstream", bufs=4) as vstream, \
             tc.tile_pool(name="wu", bufs=1) as wupool:

            qsl_sb = p2pool.tile([P, B, EC, QS], BF16)
            nc.gpsimd.dma_start(qsl_sb[:], qsl_d.ap())

            exp_all = p2pool.tile([P, B, SC, QS], BF16)
            z_sb = p2pool.tile([P, SC, QS], BF16)
            r_sb = p2pool.tile([P, SC, QS], BF16)

            kt_v = kt_d.ap().rearrange("b p (ec s) -> b p ec s", ec=EC)
            v_v = v_d.ap().rearrange("b p (st e) -> b p st e", st=SC)

            # ---- scores + exp + Z + attn, pipelined over k-halves ----
            # All scores (both halves) stream on the PE back-to-back; each
            # half's softmax join (Z tail, 1/Z on ACT, attn muls on DVE)
            # overlaps the other half's PE work.
            with tc.tile_pool(name="sps", bufs=1, space="PSUM") as spspool:
                # PE warm-up: ~5us of throwaway matmuls during the head
                # DMAs so the HAM un-throttles (1.2 -> 2.4 GHz).
                wu_a = wupool.tile([P, P], BF16)
                wu_b = wupool.tile([P, E], BF16)
                nc.vector.memset(wu_a[:], 0.0)
                nc.vector.memset(wu_b[:], 0.0)
                for i in range(24):
                    ps_w = spspool.tile([P, HC, QS], F32, tag="sps", bufs=2,
                                        name=f"ps_w{i}")
                    nc.tensor.matmul(ps_w[:, 0, :E], wu_a[:], wu_b[:],
                                     start=True, stop=True)

                for half in range(2):
                    for b in range(B):
                        kt_h = kstream.tile([P, EC, HS], BF16, tag="kt",
                                            name=f"kt_{half}_{b}")
                        nc.sync.dma_start(
                            kt_h[:],
                            kt_v[b, :, :, half * HS:(half + 1) * HS])
                        ps_s = spspool.tile([P, HC, QS], F32, tag="sps",
                                            bufs=2, name=f"ps_s{half}_{b}")
                        for kc8 in range(HC):
                            for ec in range(EC):
                                mm(ps_s[:, kc8, :],
                                   kt_h[:, ec, kc8 * P:(kc8 + 1) * P],
                                   qsl_sb[:, b, ec, :],
                                   start=(ec == 0), stop=(ec == EC - 1))
                        nc.scalar.activation(
                            exp_all[:, b, half * HC:(half + 1) * HC, :],
                            ps_s[:],
                            mybir.ActivationFunctionType.Exp,
                            scale=SCALE)
                        zh = z_sb[:, half * HC:(half + 1) * HC, :]
                        eh = exp_all[:, b, half * HC:(half + 1) * HC, :]
                        if b == 0:
                            nc.vector.tensor_copy(zh, eh)
                        else:
                            nc.vector.tensor_add(zh, zh, eh)

                    # 1/Z = exp(-ln Z) on ScalarE (DVE RECIPROCAL is
                    # 25.8us on this tile), then attn = exp * R on DVE
                    rh = r_sb[:, half * HC:(half + 1) * HC, :]
                    nc.scalar.activation(
                        rh, z_sb[:, half * HC:(half + 1) * HC, :],
                        mybir.ActivationFunctionType.Ln)
                    nc.scalar.activation(
                        rh, rh, mybir.ActivationFunctionType.Exp,
                        scale=-1.0)
                    for b in range(B):
                        eh = exp_all[:, b, half * HC:(half + 1) * HC, :]
                        nc.vector.tensor_mul(eh, eh, rh)

            # ---- combine: out = attn^T @ v ----
            # runs after all scores/joins, so each (b, qc) accumulates all
            # 16 k-chunks in one PSUM group; output copies once via DVE
            with tc.tile_pool(name="ops", bufs=1, space="PSUM") as opspool, \
                 tc.tile_pool(name="outp", bufs=4) as outpool:
                for b in range(B):
                    v_b = vstream.tile([P, SC, E], BF16, tag="v",
                                       name=f"v_{b}")
                    nc.sync.dma_start(v_b[:], v_v[b])
                    for qc in range(2):
                        ps_o = opspool.tile([P, E], F32, tag="ops",
                                            bufs=8, name=f"ps_o{b}_{qc}")
                        for st in range(SC):
                            nc.tensor.matmul(
                                ps_o[:],
                                exp_all[:, b, st, qc * P:(qc + 1) * P],
                                v_b[:, st, :],
                                start=(st == 0), stop=(st == SC - 1))
                        o_sb = outpool.tile([P, E], F32, tag="osb",
                                            name=f"o_sb{b}_{qc}")
                        nc.vector.tensor_copy(o_sb[:], ps_o[:])
                        nc.sync.dma_start(
                            out_d.ap()[b, qc * P:(qc + 1) * P, :], o_sb[:])

    nc.compile()
    return nc


_CACHE = {}


def get_nc(which):
    if which not in _CACHE:
        _CACHE[which] = build_nc_a() if which == "a" else build_nc_b()
    return _CACHE[which]


def make_in_maps_a(x, Wq, bq, Wk, bk, Wv, bv):
    ones = np.ones((1, P), np.float32)
    maps = []
    for c in range(N_CORES):
        xt = np.ascontiguousarray(x[c].T)
        maps.append({"xt": xt, "wq": Wq, "wk": Wk, "wv": Wv,
                     "bq": bq, "bk": bk, "bv": bv, "onesv": ones})
    return maps


def make_in_maps_b(res_a):
    ktall = np.stack([res_a[b]["kt"] for b in range(B)])  # [B,P,EC*S]
    vall = np.stack([res_a[b]["v"] for b in range(B)])
    # qt_b [P, EC*S] -> [P, EC, S]; core c needs columns c*QS..
    qts = [res_a[b]["qt"].reshape(P, EC, S) for b in range(B)]
    maps = []
    for c in range(N_CORES):
        qsl = np.stack([q[:, :, c * QS:(c + 1) * QS] for q in qts],
                       axis=1)  # [P, B, EC, QS]
        maps.append({"ktall": ktall, "vall": vall,
                     "qsl": np.ascontiguousarray(qsl)})
    return maps


def run(x, Wq, bq, Wk, bk, Wv, bv, trace=False):
    nc_a = get_nc("a")
    nc_b = get_nc("b")
    ra = bass_utils.run_bass_kernel_spmd(
        nc_a, make_in_maps_a(x, Wq, bq, Wk, bk, Wv, bv),
        core_ids=list(range(N_CORES)), trace=trace)
    rb = bass_utils.run_bass_kernel_spmd(
        nc_b, make_in_maps_b(ra.results),
        core_ids=list(range(N_CORES)), trace=trace)
    out = np.empty((B, S, E), np.float32)
    for c in range(N_CORES):
        out[:, c * QS:(c + 1) * QS, :] = rb.results[c]["out"]
    return out, ra, rb


def kernel(x, Wq, bq, Wk, bk, Wv, bv):
    out, _, _ = run(np.asarray(x, np.float32),
                    np.asarray(Wq, np.float32), np.asarray(bq, np.float32),
                    np.asarray(Wk, np.float32), np.asarray(bk, np.float32),
                    np.asarray(Wv, np.float32), np.asarray(bv, np.float32))
    return out


# revision 24
# speedup vs baseline: 1.0705x; 1.0001x over previous
"""Trainium2 Bass kernel for batch-axis-softmax attention (8 NeuronCores).

Reference computation (B=8, S=2048, D_IN=512, D_OUT=256):
    q = relu(x @ Wq + bq); k = relu(x @ Wk + bk); v = relu(x @ Wv + bv)
    scores = q @ k^T / sqrt(256)            # [B, S, S]
    attn = softmax(scores, axis=0)          # softmax over the BATCH axis
    out = attn @ v                          # [B, S, D_OUT]

Because the softmax runs over the batch axis, every (q, k) position needs
all 8 batches' scores. Two SPMD launches, no collectives:

  Launch A (batch-parallel): core b computes k^T, q^T (both [E, S]) and
  v ([S, E]) for batch b in float32r matmuls (full PE rate, near-fp32),
  emitting bf16. x arrives host-pre-transposed so the contraction dim is
  on partitions.

  Host: gathers k/v of all batches (the "all-gather"), slices q columns
  per core.

  Launch B (query-parallel): core c owns query rows [c*256, (c+1)*256) of
  EVERY batch, so the batch-axis softmax is core-local. scores^T =
  k_b @ q_slice^T in bf16 (f32 PSUM), exp on ScalarE (scores are in
  [0.18, 2.2] so no max subtraction), Z = sum_b exp, 1/Z = exp(-ln Z) on
  ScalarE (DVE RECIPROCAL measures 25.8 us), attn = exp * (1/Z) on DVE,
  out = attn^T @ v. Scores/exp/Z stream per k-half so each half's
  softmax join (ACT+DVE) overlaps the other half's PE work; the combine
  then accumulates all 16 k-chunks per (batch, q-chunk) in one PSUM
  group. ~5 us of throwaway matmuls at kernel start un-throttle the PE
  clock (HAM) before real work arrives; k/v stream on the HWDGE (sync)
  DMA queue, qsl on SWDGE (gpsimd) so it cannot delay the first k tile.

exp/attn/Z/k/q/v are bf16: each elementwise bf16 error is independent
across the 2048 k positions summed by attn @ v, so it averages down by
~sqrt(2048); measured end-to-end max relative error is 2.0e-3.
"""

import numpy as np

import concourse.bacc as bacc
import concourse.mybir as mybir
import concourse.tile as tile
from concourse import bass_utils

F32 = mybir.dt.float32
F32R = mybir.dt.float32r
BF16 = mybir.dt.bfloat16

B = 8
S = 2048
D = 512
E = 256
P = 128
N_CORES = 8
QS = S // N_CORES

DC = D // P
EC = E // P
SC = S // P
SCALE = 1.0 / 16.0


def build_nc_a():
    """Projections for one batch: kt/qt [e,s] and v [s,e], all bf16."""
    nc = bacc.Bacc("TRN2", target_bir_lowering=False, debug=False,
                   num_devices=N_CORES)
    xt_d = nc.dram_tensor("xt", [D, S], F32, kind="ExternalInput")
    wq_d = nc.dram_tensor("wq", [D, E], F32, kind="ExternalInput")
    wk_d = nc.dram_tensor("wk", [D, E], F32, kind="ExternalInput")
    wv_d = nc.dram_tensor("wv", [D, E], F32, kind="ExternalInput")
    bq_d = nc.dram_tensor("bq", [E], F32, kind="ExternalInput")
    bk_d = nc.dram_tensor("bk", [E], F32, kind="ExternalInput")
    bv_d = nc.dram_tensor("bv", [E], F32, kind="ExternalInput")
    ones_d = nc.dram_tensor("onesv", [1, P], F32, kind="ExternalInput")
    kt_o = nc.dram_tensor("kt", [P, EC * S], BF16, kind="ExternalOutput")
    qt_o = nc.dram_tensor("qt", [P, EC * S], BF16, kind="ExternalOutput")
    v_o = nc.dram_tensor("v", [P, SC * E], BF16, kind="ExternalOutput")

    def mm(out, lhsT, rhs, start, stop):
        nc.tensor.matmul(out, lhsT, rhs, start=start, stop=stop)

    with tile.TileContext(nc) as tc:
        with tc.tile_pool(name="cpool", bufs=1) as cpool, \
             tc.tile_pool(name="wu", bufs=1) as wupool, \
             tc.tile_pool(name="wups", bufs=1, space="PSUM") as wups, \
             tc.tile_pool(name="p1", bufs=1) as p1pool, \
             tc.tile_pool(name="p1ps", bufs=1, space="PSUM") as p1ps:
            wu_a = wupool.tile([P, P], BF16)
            wu_b = wupool.tile([P, 512], BF16)
            nc.vector.memset(wu_a[:], 0.0)
            nc.vector.memset(wu_b[:], 0.0)
            ps_w = wups.tile([P, 512], F32)
            for i in range(24):
                nc.tensor.matmul(ps_w[:], wu_a[:], wu_b[:],
                                 start=True, stop=True)
            wq_sb = cpool.tile([P, DC, E], F32R)
            wk_sb = cpool.tile([P, DC, E], F32R)
            wv_sb = cpool.tile([P, DC, E], F32R)
            bq_sb = cpool.tile([P, EC], F32)
            bk_sb = cpool.tile([P, EC], F32)
            bv_row = cpool.tile([1, E], F32R)
            ones_row = cpool.tile([1, P], F32R)
            xt_sb = p1pool.tile([P, DC, S], F32R)
            # k weights first, then x^T chunks: the first k matmul can
            # start ~7us in instead of waiting for all 6 MB of constants
            nc.sync.dma_start(wk_sb[:], wk_d.ap().rearrange(
                "(dc p) e -> p dc e", p=P).bitcast(F32R))
            nc.sync.dma_start(bk_sb[:], bk_d.ap().rearrange(
                "(ec p) -> p ec", p=P))
            xt_r = xt_d.ap().rearrange(
                "(dc p) s -> p dc s", p=P).bitcast(F32R)
            for dc in range(DC):
                nc.sync.dma_start(xt_sb[:, dc, :], xt_r[:, dc, :])
            nc.sync.dma_start(wq_sb[:], wq_d.ap().rearrange(
                "(dc p) e -> p dc e", p=P).bitcast(F32R))
            nc.sync.dma_start(bq_sb[:], bq_d.ap().rearrange(
                "(ec p) -> p ec", p=P))
            nc.sync.dma_start(wv_sb[:], wv_d.ap().rearrange(
                "(dc p) e -> p dc e", p=P).bitcast(F32R))
            nc.sync.dma_start(bv_row[:], bv_d.ap().rearrange(
                "(a e) -> a e", a=1).bitcast(F32R))
            nc.sync.dma_start(ones_row[:], ones_d.ap().bitcast(F32R))

            # kt / qt: [e, s] = relu(W^T @ x^T + b)
            for w_sb, b_sb, o_d, nm in ((wk_sb, bk_sb, kt_o, "k"),
                                        (wq_sb, bq_sb, qt_o, "q")):
                t_sb = p1pool.tile([P, EC, S], BF16, name=f"t_{nm}")
                for ec in range(EC):
                    for sh in range(2):
                        ps_k = p1ps.tile([P, 1024], F32, tag="kps", bufs=2,
                                         name=f"ps_{nm}{ec}{sh}")
                        for dc in range(DC):
                            for st in range(2):
                                mm(ps_k[:, st * 512:(st + 1) * 512],
                                   w_sb[:, dc, ec * P:(ec + 1) * P],
                                   xt_sb[:, dc,
                                         sh * 1024 + st * 512:
                                         sh * 1024 + (st + 1) * 512],
                                   start=(dc == 0), stop=(dc == DC - 1))
                        nc.scalar.activation(
                            t_sb[:, ec, sh * 1024:(sh + 1) * 1024],
                            ps_k[:],
                            mybir.ActivationFunctionType.Relu,
                            bias=b_sb[:, ec:ec + 1])
                        nc.sync.dma_start(
                            o_d.ap().rearrange(
                                "p (ec s) -> p ec s", ec=EC)
                            [:, ec, sh * 1024:(sh + 1) * 1024],
                            t_sb[:, ec, sh * 1024:(sh + 1) * 1024])

            # v: [s, e] = relu(x @ Wv + bv), bias via rank-1 matmul
            v_sb = p1pool.tile([P, SC * E], BF16)
            for sp in range(SC // 2):
                ps_v = p1ps.tile([P, 2 * E], F32, tag="vps", bufs=2)
                for half in range(2):
                    st = sp * 2 + half
                    sl = ps_v[:, half * E:(half + 1) * E]
                    mm(sl, ones_row[0:1, :], bv_row[0:1, :],
                       start=True, stop=False)
                    for dc in range(DC):
                        mm(sl, xt_sb[:, dc, st * P:(st + 1) * P],
                           wv_sb[:, dc, :],
                           start=False, stop=(dc == DC - 1))
                nc.scalar.activation(
                    v_sb[:, sp * 2 * E:(sp + 1) * 2 * E], ps_v[:],
                    mybir.ActivationFunctionType.Relu)
                nc.sync.dma_start(
                    v_o.ap()[:, sp * 2 * E:(sp + 1) * 2 * E],
                    v_sb[:, sp * 2 * E:(sp + 1) * 2 * E])

    nc.compile()
    return nc


def build_nc_b():
    """Attention for one q-slice of 256 rows, all batches.

    Pipelined over the two k-halves: while half 1's scores stream on the
    PE, half 0's Z/R/attn (ACT+DVE) and combine run, and vice versa —
    the softmax join never idles the PE. Output accumulates in SBUF f32
    across the two halves.
    """
    nc = bacc.Bacc("TRN2", target_bir_lowering=False, debug=False,
                   num_devices=N_CORES)
    kt_d = nc.dram_tensor("ktall", [B, P, EC * S], BF16,
                          kind="ExternalInput")
    v_d = nc.dram_tensor("vall", [B, P, SC * E], BF16,
                         kind="ExternalInput")
    qsl_d = nc.dram_tensor("qsl", [P, B, EC, QS], BF16,
                           kind="ExternalInput")
    out_d = nc.dram_tensor("out", [B, QS, E], F32, kind="ExternalOutput")

    HS = S // 2          # 1024 columns of k per half
    HC = SC // 2         # 8 kpos chunks per half

    def mm(out, lhsT, rhs, start, stop):
        nc.tensor.matmul(out, lhsT, rhs, start=start, stop=stop)

    with tile.TileContext(nc) as tc:
        with tc.tile_pool(name="p2", bufs=1) as p2pool, \
             tc.tile_pool(name="kstream", bufs=4) as kstream, \
             tc.tile_pool(name="vstream", bufs=4) as vstream, \
             tc.tile_pool(name="wu", bufs=1) as wupool:

            qsl_sb = p2pool.tile([P, B, EC, QS], BF16)
            nc.gpsimd.dma_start(qsl_sb[:], qsl_d.ap())

            exp_all = p2pool.tile([P, B, SC, QS], BF16)
            z_sb = p2pool.tile([P, SC, QS], BF16)
            r_sb = p2pool.tile([P, SC, QS], BF16)

            kt_v = kt_d.ap().rearrange("b p (ec s) -> b p ec s", ec=EC)
            v_v = v_d.ap().rearrange("b p (st e) -> b p st e", st=SC)

            # ---- scores + exp + Z + attn, pipelined over k-halves ----
            # All scores (both halves) stream on the PE back-to-back; each
            # half's softmax join (Z tail, 1/Z on ACT, attn muls on DVE)
            # overlaps the other half's PE work.
            with tc.tile_pool(name="sps", bufs=1, space="PSUM") as spspool:
                # PE warm-up: ~5us of throwaway matmuls during the head
                # DMAs so the HAM un-throttles (1.2 -> 2.4 GHz).
                wu_a = wupool.tile([P, P], BF16)
                wu_b = wupool.tile([P, E], BF16)
                nc.vector.memset(wu_a[:], 0.0)
                nc.vector.memset(wu_b[:], 0.0)
                for i in range(24):
                    ps_w = spspool.tile([P, HC, QS], F32, tag="sps", bufs=2,
                                        name=f"ps_w{i}")
                    nc.tensor.matmul(ps_w[:, 0, :E], wu_a[:], wu_b[:],
                                     start=True, stop=True)

                for half in range(2):
                    for b in range(B):
                        kt_h = kstream.tile([P, EC, HS], BF16, tag="kt",
                                            name=f"kt_{half}_{b}")
                        nc.sync.dma_start(
                            kt_h[:],
                            kt_v[b, :, :, half * HS:(half + 1) * HS])
                        ps_s = spspool.tile([P, HC, QS], F32, tag="sps",
                                            bufs=2, name=f"ps_s{half}_{b}")
                        for kc8 in range(HC):
                            for ec in range(EC):
                                mm(ps_s[:, kc8, :],
                                   kt_h[:, ec, kc8 * P:(kc8 + 1) * P],
                                   qsl_sb[:, b, ec, :],
                                   start=(ec == 0), stop=(ec == EC - 1))
                        nc.scalar.activation(
                            exp_all[:, b, half * HC:(half + 1) * HC, :],
                            ps_s[:],
                            mybir.ActivationFunctionType.Exp,
                            scale=SCALE)
                        zh = z_sb[:, half * HC:(half + 1) * HC, :]
                        eh = exp_all[:, b, half * HC:(half + 1) * HC, :]
                        if b == 0:
                            nc.vector.tensor_copy(zh, eh)
                        else:
                            nc.vector.tensor_add(zh, zh, eh)

                    # 1/Z = exp(-ln Z) on ScalarE (DVE RECIPROCAL is
                    # 25.8us on this tile), then attn = exp * R on DVE
                    rh = r_sb[:, half * HC:(half + 1) * HC, :]
                    nc.scalar.activation(
                        rh, z_sb[:, half * HC:(half + 1) * HC, :],
                        mybir.ActivationFunctionType.Ln)
                    nc.scalar.activation(
                        rh, rh, mybir.ActivationFunctionType.Exp,
                        scale=-1.0)
                    for b in range(B):
                        eh = exp_all[:, b, half * HC:(half + 1) * HC, :]
                        nc.vector.tensor_mul(eh, eh, rh)

            # ---- combine: out = attn^T @ v ----
            # runs after all scores/joins, so each (b, qc) accumulates all
            # 16 k-chunks in one PSUM group; output copies once via DVE
            with tc.tile_pool(name="ops", bufs=1, space="PSUM") as opspool, \
                 tc.tile_pool(name="outp", bufs=4) as outpool:
                for b in range(B):
                    v_b = vstream.tile([P, SC, E], BF16, tag="v",
                                       name=f"v_{b}")
                    nc.sync.dma_start(v_b[:], v_v[b])
                    for qc in range(2):
                        ps_o = opspool.tile([P, E], F32, tag="ops",
                                            bufs=8, name=f"ps_o{b}_{qc}")
                        for st in range(SC):
                            nc.tensor.matmul(
                                ps_o[:],
                                exp_all[:, b, st, qc * P:(qc + 1) * P],
                                v_b[:, st, :],
                                start=(st == 0), stop=(st == SC - 1))
                        o_sb = outpool.tile([P, E], F32, tag="osb",
                                            name=f"o_sb{b}_{qc}")
                        nc.vector.tensor_copy(o_sb[:], ps_o[:])
                        nc.sync.dma_start(
                            out_d.ap()[b, qc * P:(qc + 1) * P, :], o_sb[:])

    nc.compile()
    return nc


_CACHE = {}


def get_nc(which):
    if which not in _CACHE:
        _CACHE[which] = build_nc_a() if which == "a" else build_nc_b()
    return _CACHE[which]


def make_in_maps_a(x, Wq, bq, Wk, bk, Wv, bv):
    ones = np.ones((1, P), np.float32)
    maps = []
    for c in range(N_CORES):
        xt = np.ascontiguousarray(x[c].T)
        maps.append({"xt": xt, "wq": Wq, "wk": Wk, "wv": Wv,
                     "bq": bq, "bk": bk, "bv": bv, "onesv": ones})
    return maps


def make_in_maps_b(res_a):
    ktall = np.stack([res_a[b]["kt"] for b in range(B)])  # [B,P,EC*S]
    vall = np.stack([res_a[b]["v"] for b in range(B)])
    # qt_b [P, EC*S] -> [P, EC, S]; core c needs columns c*QS..
    qts = [res_a[b]["qt"].reshape(P, EC, S) for b in range(B)]
    maps = []
    for c in range(N_CORES):
        qsl = np.stack([q[:, :, c * QS:(c + 1) * QS] for q in qts],
                       axis=1)  # [P, B, EC, QS]
        maps.append({"ktall": ktall, "vall": vall,
                     "qsl": np.ascontiguousarray(qsl)})
    return maps


def run(x, Wq, bq, Wk, bk, Wv, bv, trace=False):
    nc_a = get_nc("a")
    nc_b = get_nc("b")
    ra = bass_utils.run_bass_kernel_spmd(
        nc_a, make_in_maps_a(x, Wq, bq, Wk, bk, Wv, bv),
        core_ids=list(range(N_CORES)), trace=trace)
    rb = bass_utils.run_bass_kernel_spmd(
        nc_b, make_in_maps_b(ra.results),
        core_ids=list(range(N_CORES)), trace=trace)
    out = np.empty((B, S, E), np.float32)
    for c in range(N_CORES):
        out[:, c * QS:(c + 1) * QS, :] = rb.results[c]["out"]
    return out, ra, rb


def kernel(x, Wq, bq, Wk, bk, Wv, bv):
    out, _, _ = run(np.asarray(x, np.float32),
                    np.asarray(Wq, np.float32), np.asarray(bq, np.float32),
                    np.asarray(Wk, np.float32), np.asarray(bk, np.float32),
                    np.asarray(Wv, np.float32), np.asarray(bv, np.float32))
    return out
